# revision 39
# baseline (speedup 1.0000x reference)
"""Masked multi-head attention (sparse_attention) Trainium2 Bass kernel.

Data-parallel over batch: B=8 batch elements, one per NeuronCore.
Per-core computation for batch element b (all shapes hardcoded):
  x [1024,768], adj [1024,1024], Wq/Wk/Wv [768,768], bq/bk/bv [768], beta []
  q = x@Wq+bq; k = x@Wk+bk; v = x@Wv+bv      (12 heads of 64)
  S = q k^T / 8 + beta*adj ; masked where adj<=0 ; P = softmax(S)
  out = P v  -> [1024, 768]

Design (v3, ~195us vs 242us phase-serial baseline):

1. Single fused software pipeline over head pairs.  The ACT engine owns
   the 12.6M-element exp (~92us at 1 elem/cycle/lane) and must run the
   whole kernel, not just a trailing attention phase.  Attention for
   pair c starts as soon as qt[c]/kt[c] exist (~t=15us); V-projection,
   late QK chunks, masks, finalize and output DMA are dribbled into the
   per-k weave so PE/DVE/DMA stay busy under the ACT/PE roofline.
   Weave per block c: step k emits S(c,k)+exp+mask-mul, then one PV
   quarter of pair c-1 (k<4) or two finalize q-chunks of pair c-1 plus
   one QK(c+1) projection unit (k>=4).  The last pair's q-half-0 PV
   chains are fed incrementally during block 5 and its fins interleave
   the q-half-1 pieces, shortening the pipeline drain.

2. bf16 inputs, cast on host.  The kernel computes in bf16 anyway;
   shipping x/adj/W as bf16 halves input HBM traffic (13.75 -> 6.9 MB,
   the startup wall is DMA-bandwidth-bound) and removes all SWDGE
   casting DMAs.  DMA delivery order matches consumption: x, Wq, Wk,
   adj half-0 (masks 0-3), Wv, adj half-1.  Output is bf16 too, cast
   back on host; rel err ~6.4e-3 (vs 5.7e-3 all-f32-io).

3. Per-core dataflow (per head pair c = heads 2c, 2c+1):
   X^T via PE transposes (bf16, psum) evacuated to SBUF by ACT/DVE;
   Q^T,K^T,V = bf16 matmuls.  m^T[k] = (adjT>0)*exp(beta*adjT) via PE
   transposes of adj column slices.  S^T pair = K_h @ Q_h^T as two
   row-tiled K=64 matmuls (tile_position row groups, emitted adjacent;
   they launch concurrently when ACT has freed both psum slots).
   P^T = m^T * exp(S^T/8) (ACT exp + DVE mask-mul, both bf16).
   out^T[65,512] = [V_h|1]^T @ P^T -- the ones column emits softmax
   row-sums for free.  PE-transpose back per q-chunk, scale by
   1/rowsum, DMA out in thirds (after pairs 2, 4, 5).

   PSUM: 2 banks transient work ring (proj/masks/finalize), 4 banks
   S ring (2 x [128,1024]), 2 banks PV accumulators.  P tiles for
   k<4 are double-buffered so pair c+1's mask-muls decouple from
   pair c's PV reads (deadlock- and stall-avoidance in the weave).

Rejected by measurement: fp8 S/PV (logit error ~5% > 2e-2 budget),
PV column-packing with M=1 rowsum chains (col tiling disables fast
weight load; +46us), adj column-strip DMA (512B-descriptor storm on
the sync queue), [128,512] S tiles with 4-5 deep rings (+10us ACT
overhead beats the concurrency gain), full P double-buffering (+34us,
scheduler pathology).
"""

import sys

import numpy as np

try:
    import concourse.bass as bass
except ImportError:  # container default location
    sys.path.insert(0, "/opt/trn_rl_repo")
    import concourse.bass as bass

from contextlib import ExitStack

import concourse.bacc as bacc
import concourse.mybir as mybir
import concourse.tile as tile
from concourse.bass_utils import run_bass_kernel_spmd
from concourse.masks import make_identity

B, N, D, H = 8, 1024, 768, 12
HD = 64
P = 128
NT = N // P  # 8 row chunks
DT = D // P  # 6 feature chunks
NH = 512  # free-dim tile for matmuls
HD1 = HD + 1  # head dim + ones column
NPAIR = H // 2  # 6 head pairs

F32 = mybir.dt.float32
BF16 = mybir.dt.bfloat16
AF = mybir.ActivationFunctionType
ALU = mybir.AluOpType


def _emit(tc, ctx, x_d, adj_d, w_d, b_d, beta_d, out_d):
    nc = tc.nc

    const = ctx.enter_context(tc.tile_pool(name="const", bufs=1))
    ident = const.tile([P, P], F32, tag="ident")
    make_identity(nc, ident)
    identb = const.tile([P, P], BF16, tag="identb")
    make_identity(nc, identb)

    def bcast(ap, n_part):
        return bass.AP(tensor=ap.tensor, offset=ap.offset, ap=[[0, n_part]] + list(ap.ap))

    beta_sb = const.tile([P, 1], F32, tag="beta")
    nc.gpsimd.dma_start(out=beta_sb, in_=bcast(beta_d[0], P))
    bq_sb = const.tile([P, DT], F32, tag="bq")
    bk_sb = const.tile([P, DT], F32, tag="bk")
    bv_bc = const.tile([P, D], F32, tag="bv")

    # ---- persistent tensors ----
    pers = ctx.enter_context(tc.tile_pool(name="pers", bufs=1))
    w_sb = {}
    for wname in ("wq", "wk", "wv"):
        w_sb[wname] = [
            pers.tile([P, D], BF16, tag=f"{wname}{c}", name=f"{wname}{c}")
            for c in range(DT)
        ]
    xt = [pers.tile([P, N], BF16, tag=f"xt{c}", name=f"xt{c}") for c in range(DT)]
    qt = [pers.tile([P, N], BF16, tag=f"qt{c}", name=f"qt{c}") for c in range(DT)]
    kt = [pers.tile([P, N], BF16, tag=f"kt{c}", name=f"kt{c}") for c in range(DT)]
    v_sb = [pers.tile([P, H * HD1], BF16, tag=f"v{i}", name=f"v{i}") for i in range(NT)]
    m_sb = [pers.tile([P, N], BF16, tag=f"m{k}", name=f"m{k}") for k in range(NT)]
    out_sb = [pers.tile([P, D], BF16, tag=f"os{i}", name=f"os{i}") for i in range(NT)]

    # ---- input DMAs (issue everything up front; engines overlap) ----
    # adjp outlives xp: create it first so closing xp pops the alloc stack.
    adjp = ctx.enter_context(tc.tile_pool(name="adjp", bufs=1))
    es_x = ExitStack()
    xp = es_x.enter_context(tc.tile_pool(name="xp", bufs=1))
    x_sb = [xp.tile([P, D], BF16, tag=f"x{i}", name=f"x{i}") for i in range(NT)]
    for i in range(NT):
        nc.sync.dma_start(out=x_sb[i], in_=x_d[i * P:(i + 1) * P, :])

    # biases on the (otherwise idle) gpsimd queue
    nc.gpsimd.dma_start(out=bq_sb, in_=b_d["bq"].rearrange("(c p) -> p c", p=P))
    nc.gpsimd.dma_start(out=bk_sb, in_=b_d["bk"].rearrange("(c p) -> p c", p=P))
    nc.gpsimd.dma_start(out=bv_bc, in_=bcast(b_d["bv"], P))

    # adj in column-half tiles: masks k<4 only gate on the first half.
    adj_sb = [
        [adjp.tile([P, NH], BF16, tag=f"aj{j}_{h}", name=f"aj{j}_{h}") for h in range(2)]
        for j in range(NT)
    ]
    # delivery order: wq, wk (gate S(0)), adj half 0 (masks 0-3), wv, adj half 1
    for wname in ("wq", "wk"):
        for c in range(DT):
            nc.sync.dma_start(
                out=w_sb[wname][c], in_=w_d[wname][c * P:(c + 1) * P, :]
            )
    for j in range(NT):
        nc.sync.dma_start(out=adj_sb[j][0], in_=adj_d[j * P:(j + 1) * P, 0:NH])
    for c in range(DT):
        nc.sync.dma_start(out=w_sb["wv"][c], in_=w_d["wv"][c * P:(c + 1) * P, :])
    for j in range(NT):
        nc.sync.dma_start(out=adj_sb[j][1], in_=adj_d[j * P:(j + 1) * P, NH:N])

    # ---- psum pools ----
    work = ctx.enter_context(tc.tile_pool(name="work", space="PSUM", bufs=2))
    pss = ctx.enter_context(tc.tile_pool(name="pss", space="PSUM", bufs=2))
    pso = ctx.enter_context(tc.tile_pool(name="pso", space="PSUM", bufs=2))

    # ---- X^T: per c, two groups of 4 PE transposes + one evac ----
    # g=0 groups first: they only need x tiles 0-3, so PE starts sooner.
    # Evacs split ACT/DVE to relieve the ACT pileup around block 0.
    for g in range(2):
        for c in range(DT):
            tp = work.tile([P, NH], BF16, tag="work", name="tp")
            for bb in range(4):
                i = g * 4 + bb
                nc.tensor.transpose(
                    tp[:, bb * P:(bb + 1) * P], x_sb[i][:, c * P:(c + 1) * P], identb
                )
            if c % 2 == g % 2:
                nc.scalar.copy(xt[c][:, g * NH:(g + 1) * NH], tp)
            else:
                nc.vector.tensor_copy(xt[c][:, g * NH:(g + 1) * NH], tp)
    es_x.close()

    # ---- masks: m^T[k] = (adjT>0)*exp(beta*adjT) ----
    etm = ctx.enter_context(tc.tile_pool(name="etm", bufs=2))

    def emit_mask(k):
        for g in range(2):
            tp = work.tile([P, NH], BF16, tag="work", name="mtp")
            for bb in range(4):
                j = g * 4 + bb
                nc.tensor.transpose(
                    tp[:, bb * P:(bb + 1) * P],
                    adj_sb[j][k // 4][:, (k % 4) * P:(k % 4 + 1) * P],
                    identb,
                )
            e_m = etm.tile([P, NH], BF16, tag="em", name="em")
            nc.scalar.activation(e_m, tp, AF.Exp, scale=beta_sb[:, 0:1])
            nc.vector.scalar_tensor_tensor(
                out=m_sb[k][:, g * NH:(g + 1) * NH],
                in0=tp,
                scalar=0.0,
                in1=e_m,
                op0=ALU.is_gt,
                op1=ALU.mult,
            )

    def emit_qk_chunk(c, wname, dst, bias_sb, qh):
        mm = work.tile([P, NH], F32, tag="work", name="mm")
        for kc in range(DT):
            nc.tensor.matmul(
                mm,
                lhsT=w_sb[wname][kc][:, c * P:(c + 1) * P],
                rhs=xt[kc][:, qh * NH:(qh + 1) * NH],
                start=(kc == 0),
                stop=(kc == DT - 1),
            )
        nc.vector.tensor_scalar_add(
            dst[c][:, qh * NH:(qh + 1) * NH], mm, bias_sb[:, c:c + 1]
        )

    def emit_v(i):
        for s, w in ((0, NH), (NH, D - NH)):
            mm = work.tile([P, NH], F32, tag="work", name="vmm")
            for kc in range(DT):
                nc.tensor.matmul(
                    mm[:, 0:w],
                    lhsT=xt[kc][:, i * P:(i + 1) * P],
                    rhs=w_sb["wv"][kc][:, s:s + w],
                    start=(kc == 0),
                    stop=(kc == DT - 1),
                )
            nh = w // HD
            h0 = s // HD
            dst3 = v_sb[i].rearrange("p (h j) -> p h j", j=HD1)[:, h0:h0 + nh, 0:HD]
            src3 = mm[:, 0:w].rearrange("p (h j) -> p h j", j=HD)
            bias3 = bv_bc[:, s:s + w].rearrange("p (h j) -> p h j", j=HD)
            nc.vector.tensor_add(dst3, src3, bias3)
        ones3 = v_sb[i].rearrange("p (h j) -> p h j", j=HD1)[:, :, HD:HD1]
        nc.vector.memset(ones3, 1.0)

    # first two QK chunks before the attention pipeline starts
    for c in (0, 1):
        for wname, dst, bias_sb in (("wq", qt, bq_sb), ("wk", kt, bk_sb)):
            for qh in range(2):
                emit_qk_chunk(c, wname, dst, bias_sb, qh)

    # ---- attention pipeline over head pairs ----
    etq = ctx.enter_context(tc.tile_pool(name="etq", bufs=8))
    pp = ctx.enter_context(tc.tile_pool(name="pp", bufs=1))
    otp = ctx.enter_context(tc.tile_pool(name="otp", bufs=1))
    fin = ctx.enter_context(tc.tile_pool(name="fin", bufs=2))

    p_gen = {}  # pair -> [sub][k] tile handles
    ot_sb = [None] * H

    def emit_s_step(c, k):
        """S + exp + mask-mul for pair c, key chunk k."""
        if k == 0:
            p_gen[c] = [[None] * NT for _ in range(2)]
        p_tiles = p_gen[c]
        sps = [pss.tile([P, N], F32, tag="s", name=f"s{sub}") for sub in range(2)]
        # adjacent different-row-group matmuls for tile concurrency
        for qh in range(2):
            for sub in range(2):
                r0 = sub * HD
                nc.tensor.matmul(
                    sps[sub][:, qh * NH:(qh + 1) * NH],
                    lhsT=kt[c][r0:r0 + HD, k * P:(k + 1) * P],
                    rhs=qt[c][r0:r0 + HD, qh * NH:(qh + 1) * NH],
                    start=True,
                    stop=True,
                    tile_position=(r0, 0),
                )
        for sub in range(2):
            e = etq.tile([P, N], BF16, tag="et", name="et")
            nc.scalar.activation(e, sps[sub], AF.Exp, scale=0.125)
            p_tiles[sub][k] = pp.tile(
                [P, N], BF16, tag=f"p{sub}_{k}", name=f"p{sub}_{k}",
                bufs=2 if k < 6 else 1,
            )
            nc.vector.tensor_mul(p_tiles[sub][k], e, m_sb[k])

    def emit_pv_piece(c, piece):
        """One (sub, qh) quarter of PV for pair c: 8 accumulating matmuls."""
        sub, qh = piece // 2, piece % 2
        h = 2 * c + sub
        if qh == 0:
            ot_sb[h] = otp.tile([HD1, N], BF16, tag=f"ot{sub}", name=f"ot{h}")
        p_tiles = p_gen[c]
        ops = pso.tile([HD1, NH], F32, tag="ov", name="ov")
        for k in range(NT):
            nc.tensor.matmul(
                ops,
                lhsT=v_sb[k][:, h * HD1:(h + 1) * HD1],
                rhs=p_tiles[sub][k][:, qh * NH:(qh + 1) * NH],
                start=(k == 0),
                stop=(k == NT - 1),
            )
        nc.vector.tensor_copy(ot_sb[h][:, qh * NH:(qh + 1) * NH], ops)
        if piece == 3:
            del p_gen[c]

    def emit_fin_qc(c, qc):
        # transpose the pair's out^T for q-chunk qc, scale by 1/rowsum, DMA out
        fp = work.tile([P, NH], BF16, tag="work", name="fp")
        for sub in range(2):
            # stride 66 keeps the bf16 psum write 4-byte aligned
            nc.tensor.transpose(
                fp[:, sub * 66:sub * 66 + HD1],
                ot_sb[2 * c + sub][:, qc * P:(qc + 1) * P],
                identb[0:HD1, 0:HD1],
            )
        fp3 = fp[:, 0:132].rearrange("p (h j) -> p h j", j=66)
        rec = fin.tile([P, 2], F32, tag="rec", name="rec")
        nc.vector.reciprocal(rec, fp3[:, :, HD:HD1].squeeze(-1))
        rec_b = bass.AP(
            tensor=rec.tensor, offset=rec.offset, ap=list(rec.ap) + [[0, HD]]
        )
        out3 = out_sb[qc].rearrange("p (h j) -> p h j", j=HD)
        nc.vector.tensor_mul(out3[:, 2 * c:2 * c + 2, :], fp3[:, :, 0:HD], rec_b)
        if c in (2, 4, 5):
            lo, hi = {2: (0, 384), 4: (384, 640), 5: (640, 768)}[c]
            nc.sync.dma_start(
                out=out_d[qc * P:(qc + 1) * P, lo:hi],
                in_=out_sb[qc][:, lo:hi],
            )

    # Last-pair PV chains for q-half 0, fed incrementally during block 5's
    # fin steps so only the q-half-1 chains remain for the drain.
    last_chains = {}

    def open_last_chains(cc):
        for sub in range(2):
            ot_sb[2 * cc + sub] = otp.tile(
                [HD1, N], BF16, tag=f"ot{sub}", name=f"ot{2 * cc + sub}"
            )
            last_chains[sub] = pso.tile([HD1, NH], F32, tag="ov", name=f"lc{sub}")

    def feed_last_chains(cc, ks):
        p_tiles = p_gen[cc]
        for sub in range(2):
            h = 2 * cc + sub
            for kk in ks:
                nc.tensor.matmul(
                    last_chains[sub],
                    lhsT=v_sb[kk][:, h * HD1:(h + 1) * HD1],
                    rhs=p_tiles[sub][kk][:, 0:NH],
                    start=(kk == 0),
                    stop=(kk == NT - 1),
                )

    def close_last_chains(cc):
        for sub in range(2):
            nc.vector.tensor_copy(ot_sb[2 * cc + sub][:, 0:NH], last_chains[sub])

    # Steady-state weave per block c:
    #   k step:  S(c,k), then
    #     c==0:  masks k+2, adj dma k+4, V chunk k
    #     c>=1:  k<4: PV piece k of pair c-1;  k>=4: finalize q-chunks of c-1
    #            plus one QK(c+1) projection unit at odd k
    for c in range(NPAIR):
        for k in range(NT):
            if c == 0:
                emit_mask(k)
            emit_s_step(c, k)
            if c == 0:
                if k >= 3:
                    emit_v(k - 3)
                if k == NT - 1:
                    for i in range(NT - 3, NT):
                        emit_v(i)
            else:
                if k < 4:
                    emit_pv_piece(c - 1, k)
                else:
                    emit_fin_qc(c - 1, 2 * (k - 4))
                    emit_fin_qc(c - 1, 2 * (k - 4) + 1)
                if k >= 4 and c + 1 < DT:
                    wname, dst, bias_sb = (
                        ("wq", qt, bq_sb) if k < 6 else ("wk", kt, bk_sb)
                    )
                    emit_qk_chunk(c + 1, wname, dst, bias_sb, k % 2)
                if c == NPAIR - 1 and k >= 4:
                    if k == 4:
                        open_last_chains(c)
                        feed_last_chains(c, range(4))
                    else:
                        feed_last_chains(c, [k - 1])
    # drain: q-half-0 fins (qc 0-3) only need the incrementally-fed chains,
    # so they overlap the q-half-1 PV pieces on the PE queue
    feed_last_chains(NPAIR - 1, [NT - 1])
    close_last_chains(NPAIR - 1)
    emit_pv_piece(NPAIR - 1, 1)
    emit_fin_qc(NPAIR - 1, 0)
    emit_fin_qc(NPAIR - 1, 1)
    emit_pv_piece(NPAIR - 1, 3)
    emit_fin_qc(NPAIR - 1, 2)
    emit_fin_qc(NPAIR - 1, 3)
    for qc in range(4, NT):
        emit_fin_qc(NPAIR - 1, qc)


def build_nc():
    nc = bacc.Bacc("TRN2", target_bir_lowering=False, debug=False, num_devices=B)
    x_d = nc.dram_tensor("x", [N, D], BF16, kind="ExternalInput").ap()
    adj_d = nc.dram_tensor("adj", [N, N], BF16, kind="ExternalInput").ap()
    w_d = {
        "wq": nc.dram_tensor("wq", [D, D], BF16, kind="ExternalInput").ap(),
        "wk": nc.dram_tensor("wk", [D, D], BF16, kind="ExternalInput").ap(),
        "wv": nc.dram_tensor("wv", [D, D], BF16, kind="ExternalInput").ap(),
    }
    b_d = {
        "bq": nc.dram_tensor("bq", [D], F32, kind="ExternalInput").ap(),
        "bk": nc.dram_tensor("bk", [D], F32, kind="ExternalInput").ap(),
        "bv": nc.dram_tensor("bv", [D], F32, kind="ExternalInput").ap(),
    }
    beta_d = nc.dram_tensor("beta", [1, 1], F32, kind="ExternalInput").ap()
    out_d = nc.dram_tensor("out", [N, D], BF16, kind="ExternalOutput").ap()
    with tile.TileContext(nc) as tc, ExitStack() as ctx:
        _emit(tc, ctx, x_d, adj_d, w_d, b_d, beta_d, out_d)
    nc.compile()
    return nc


_CACHE = {}


def _get_nc():
    if "nc" not in _CACHE:
        _CACHE["nc"] = build_nc()
    return _CACHE["nc"]


def make_in_maps(input_graph, adj, Wq, bq, Wk, bk, Wv, bv, beta):
    import ml_dtypes

    f = lambda a: np.ascontiguousarray(np.asarray(a), dtype=np.float32)
    h = lambda a: np.ascontiguousarray(np.asarray(a, dtype=np.float32)).astype(
        ml_dtypes.bfloat16
    )
    wq, wk, wv = h(Wq), h(Wk), h(Wv)
    bqa, bka, bva = f(bq), f(bk), f(bv)
    beta_a = f(beta).reshape(1, 1)
    ig, ad = h(input_graph), h(adj)
    return [
        {
            "x": ig[b], "adj": ad[b],
            "wq": wq, "wk": wk, "wv": wv,
            "bq": bqa, "bk": bka, "bv": bva,
            "beta": beta_a,
        }
        for b in range(B)
    ]


def run_hw(in_maps, **kwargs):
    nc = _get_nc()
    return run_bass_kernel_spmd(nc, in_maps, list(range(B)), **kwargs)


def kernel(input_graph, adj, Wq, bq, Wk, bk, Wv, bv, beta):
    in_maps = make_in_maps(input_graph, adj, Wq, bq, Wk, bk, Wv, bv, beta)
    res = run_hw(in_maps)
    return np.stack(
        [np.asarray(res.results[i]["out"], dtype=np.float32) for i in range(B)], axis=0
    )


# revision 40
# speedup vs baseline: 1.1926x; 1.1926x over previous
"""Masked multi-head attention (sparse_attention) Trainium2 Bass kernel.

Data-parallel over batch: B=8 batch elements, one per NeuronCore.
Per-core computation for batch element b (all shapes hardcoded):
  x [1024,768], adj [1024,1024], Wq/Wk/Wv [768,768], bq/bk/bv [768], beta []
  q = x@Wq+bq; k = x@Wk+bk; v = x@Wv+bv      (12 heads of 64)
  S = q k^T / 8 + beta*adj ; masked where adj<=0 ; P = softmax(S)
  out = P v  -> [1024, 768]

Design (v3, ~195us vs 242us phase-serial baseline):

1. Single fused software pipeline over head pairs.  The ACT engine owns
   the 12.6M-element exp (~92us at 1 elem/cycle/lane) and must run the
   whole kernel, not just a trailing attention phase.  Attention for
   pair c starts as soon as qt[c]/kt[c] exist (~t=15us); V-projection,
   late QK chunks, masks, finalize and output DMA are dribbled into the
   per-k weave so PE/DVE/DMA stay busy under the ACT/PE roofline.
   Weave per block c: step k emits S(c,k)+exp+mask-mul, then one PV
   quarter of pair c-1 (k<4) or two finalize q-chunks of pair c-1 plus
   one QK(c+1) projection unit (k>=4).  The last pair's q-half-0 PV
   chains are fed incrementally during block 5 and its fins interleave
   the q-half-1 pieces, shortening the pipeline drain.

2. bf16 inputs, cast on host.  The kernel computes in bf16 anyway;
   shipping x/adj/W as bf16 halves input HBM traffic (13.75 -> 6.9 MB,
   the startup wall is DMA-bandwidth-bound) and removes all SWDGE
   casting DMAs.  DMA delivery order matches consumption: x, Wq, Wk,
   adj half-0 (masks 0-3), Wv, adj half-1.  Output is bf16 too, cast
   back on host; rel err ~6.4e-3 (vs 5.7e-3 all-f32-io).

3. Per-core dataflow (per head pair c = heads 2c, 2c+1):
   X^T via PE transposes (bf16, psum) evacuated to SBUF by ACT/DVE;
   Q^T,K^T,V = bf16 matmuls.  m^T[k] = (adjT>0)*exp(beta*adjT) via PE
   transposes of adj column slices.  S^T pair = K_h @ Q_h^T as two
   row-tiled K=64 matmuls (tile_position row groups, emitted adjacent;
   they launch concurrently when ACT has freed both psum slots).
   P^T = m^T * exp(S^T/8) (ACT exp + DVE mask-mul, both bf16).
   out^T[65,512] = [V_h|1]^T @ P^T -- the ones column emits softmax
   row-sums for free.  PE-transpose back per q-chunk, scale by
   1/rowsum, DMA out in thirds (after pairs 2, 4, 5).

   PSUM: 2 banks transient work ring (proj/masks/finalize), 4 banks
   S ring (2 x [128,1024]), 2 banks PV accumulators.  P tiles for
   k<4 are double-buffered so pair c+1's mask-muls decouple from
   pair c's PV reads (deadlock- and stall-avoidance in the weave).

Rejected by measurement: fp8 S/PV (logit error ~5% > 2e-2 budget),
PV column-packing with M=1 rowsum chains (col tiling disables fast
weight load; +46us), adj column-strip DMA (512B-descriptor storm on
the sync queue), [128,512] S tiles with 4-5 deep rings (+10us ACT
overhead beats the concurrency gain), full P double-buffering (+34us,
scheduler pathology).
"""

import sys

import numpy as np

try:
    import concourse.bass as bass
except ImportError:  # container default location
    sys.path.insert(0, "/opt/trn_rl_repo")
    import concourse.bass as bass

from contextlib import ExitStack

import concourse.bacc as bacc
import concourse.mybir as mybir
import concourse.tile as tile
from concourse.bass_utils import run_bass_kernel_spmd
from concourse.masks import make_identity

B, N, D, H = 8, 1024, 768, 12
HD = 64
P = 128
NT = N // P  # 8 row chunks
DT = D // P  # 6 feature chunks
NH = 512  # free-dim tile for matmuls
HD1 = HD + 1  # head dim + ones column
NPAIR = H // 2  # 6 head pairs

F32 = mybir.dt.float32
BF16 = mybir.dt.bfloat16
AF = mybir.ActivationFunctionType
ALU = mybir.AluOpType


def _emit(tc, ctx, x_d, adj_d, w_d, b_d, beta_d, out_d):
    nc = tc.nc

    const = ctx.enter_context(tc.tile_pool(name="const", bufs=1))
    ident = const.tile([P, P], F32, tag="ident")
    make_identity(nc, ident)
    identb = const.tile([P, P], BF16, tag="identb")
    make_identity(nc, identb)

    def bcast(ap, n_part):
        return bass.AP(tensor=ap.tensor, offset=ap.offset, ap=[[0, n_part]] + list(ap.ap))

    beta_sb = const.tile([P, 1], F32, tag="beta")
    nc.gpsimd.dma_start(out=beta_sb, in_=bcast(beta_d[0], P))
    bq_sb = const.tile([P, DT], F32, tag="bq")
    bk_sb = const.tile([P, DT], F32, tag="bk")
    bv_bc = const.tile([P, D], F32, tag="bv")

    # ---- persistent tensors ----
    pers = ctx.enter_context(tc.tile_pool(name="pers", bufs=1))
    w_sb = {}
    for wname in ("wq", "wk", "wv"):
        w_sb[wname] = [
            pers.tile([P, D], BF16, tag=f"{wname}{c}", name=f"{wname}{c}")
            for c in range(DT)
        ]
    xt = [pers.tile([P, N], BF16, tag=f"xt{c}", name=f"xt{c}") for c in range(DT)]
    qt = [pers.tile([P, N], BF16, tag=f"qt{c}", name=f"qt{c}") for c in range(DT)]
    kt = [pers.tile([P, N], BF16, tag=f"kt{c}", name=f"kt{c}") for c in range(DT)]
    v_sb = [pers.tile([P, H * HD1], BF16, tag=f"v{i}", name=f"v{i}") for i in range(NT)]
    m_sb = [pers.tile([P, N], BF16, tag=f"m{k}", name=f"m{k}") for k in range(NT)]
    out_sb = [pers.tile([P, D], BF16, tag=f"os{i}", name=f"os{i}") for i in range(NT)]

    # ---- input DMAs (issue everything up front; engines overlap) ----
    # adjp outlives xp: create it first so closing xp pops the alloc stack.
    adjp = ctx.enter_context(tc.tile_pool(name="adjp", bufs=1))
    es_x = ExitStack()
    xp = es_x.enter_context(tc.tile_pool(name="xp", bufs=1))
    x_sb = [xp.tile([P, D], BF16, tag=f"x{i}", name=f"x{i}") for i in range(NT)]
    for i in range(NT):
        nc.sync.dma_start(out=x_sb[i], in_=x_d[i * P:(i + 1) * P, :])

    # biases on the (otherwise idle) gpsimd queue
    nc.gpsimd.dma_start(out=bq_sb, in_=b_d["bq"].rearrange("(c p) -> p c", p=P))
    nc.gpsimd.dma_start(out=bk_sb, in_=b_d["bk"].rearrange("(c p) -> p c", p=P))
    nc.gpsimd.dma_start(out=bv_bc, in_=bcast(b_d["bv"], P))

    # adj in column-half tiles: masks k<4 only gate on the first half.
    adj_sb = [
        [adjp.tile([P, NH], BF16, tag=f"aj{j}_{h}", name=f"aj{j}_{h}") for h in range(2)]
        for j in range(NT)
    ]
    # delivery order: wq, wk (gate S(0)), adj half 0 (masks 0-3), wv, adj half 1
    for wname in ("wq", "wk"):
        for c in range(DT):
            nc.sync.dma_start(
                out=w_sb[wname][c], in_=w_d[wname][c * P:(c + 1) * P, :]
            )
    for j in range(NT):
        nc.sync.dma_start(out=adj_sb[j][0], in_=adj_d[j * P:(j + 1) * P, 0:NH])
    for c in range(DT):
        nc.sync.dma_start(out=w_sb["wv"][c], in_=w_d["wv"][c * P:(c + 1) * P, :])
    for j in range(NT):
        nc.sync.dma_start(out=adj_sb[j][1], in_=adj_d[j * P:(j + 1) * P, NH:N])

    # ---- psum pools ----
    work = ctx.enter_context(tc.tile_pool(name="work", space="PSUM", bufs=2))
    pss = ctx.enter_context(tc.tile_pool(name="pss", space="PSUM", bufs=2))
    pso = ctx.enter_context(tc.tile_pool(name="pso", space="PSUM", bufs=2))

    # ---- X^T: per c, two groups of 4 PE transposes + one evac ----
    # g=0 groups first: they only need x tiles 0-3, so PE starts sooner.
    # Evacs split ACT/DVE to relieve the ACT pileup around block 0.
    for g in range(2):
        for c in range(DT):
            tp = work.tile([P, NH], BF16, tag="work", name="tp")
            for bb in range(4):
                i = g * 4 + bb
                nc.tensor.transpose(
                    tp[:, bb * P:(bb + 1) * P], x_sb[i][:, c * P:(c + 1) * P], identb
                )
            if c % 2 == g % 2:
                nc.scalar.copy(xt[c][:, g * NH:(g + 1) * NH], tp)
            else:
                nc.vector.tensor_copy(xt[c][:, g * NH:(g + 1) * NH], tp)
    es_x.close()

    # ---- masks: m^T[k] = (adjT>0)*exp(beta*adjT) ----
    etm = ctx.enter_context(tc.tile_pool(name="etm", bufs=2))

    def emit_mask(k):
        for g in range(2):
            tp = work.tile([P, NH], BF16, tag="work", name="mtp")
            for bb in range(4):
                j = g * 4 + bb
                nc.tensor.transpose(
                    tp[:, bb * P:(bb + 1) * P],
                    adj_sb[j][k // 4][:, (k % 4) * P:(k % 4 + 1) * P],
                    identb,
                )
            e_m = etm.tile([P, NH], BF16, tag="em", name="em")
            nc.scalar.activation(e_m, tp, AF.Exp, scale=beta_sb[:, 0:1])
            nc.vector.scalar_tensor_tensor(
                out=m_sb[k][:, g * NH:(g + 1) * NH],
                in0=tp,
                scalar=0.0,
                in1=e_m,
                op0=ALU.is_gt,
                op1=ALU.mult,
            )

    def emit_qk_chunk(c, wname, dst, bias_sb, qh):
        mm = work.tile([P, NH], F32, tag="work", name="mm")
        for kc in range(DT):
            nc.tensor.matmul(
                mm,
                lhsT=w_sb[wname][kc][:, c * P:(c + 1) * P],
                rhs=xt[kc][:, qh * NH:(qh + 1) * NH],
                start=(kc == 0),
                stop=(kc == DT - 1),
            )
        nc.vector.tensor_scalar_add(
            dst[c][:, qh * NH:(qh + 1) * NH], mm, bias_sb[:, c:c + 1]
        )

    def emit_v(i):
        for s, w in ((0, NH), (NH, D - NH)):
            mm = work.tile([P, NH], F32, tag="work", name="vmm")
            for kc in range(DT):
                nc.tensor.matmul(
                    mm[:, 0:w],
                    lhsT=xt[kc][:, i * P:(i + 1) * P],
                    rhs=w_sb["wv"][kc][:, s:s + w],
                    start=(kc == 0),
                    stop=(kc == DT - 1),
                )
            nh = w // HD
            h0 = s // HD
            dst3 = v_sb[i].rearrange("p (h j) -> p h j", j=HD1)[:, h0:h0 + nh, 0:HD]
            src3 = mm[:, 0:w].rearrange("p (h j) -> p h j", j=HD)
            bias3 = bv_bc[:, s:s + w].rearrange("p (h j) -> p h j", j=HD)
            nc.vector.tensor_add(dst3, src3, bias3)
        ones3 = v_sb[i].rearrange("p (h j) -> p h j", j=HD1)[:, :, HD:HD1]
        nc.vector.memset(ones3, 1.0)

    # first two QK chunks before the attention pipeline starts
    for c in (0, 1):
        for wname, dst, bias_sb in (("wq", qt, bq_sb), ("wk", kt, bk_sb)):
            for qh in range(2):
                emit_qk_chunk(c, wname, dst, bias_sb, qh)

    # ---- attention pipeline over head pairs ----
    etq = ctx.enter_context(tc.tile_pool(name="etq", bufs=8))
    pp = ctx.enter_context(tc.tile_pool(name="pp", bufs=1))
    otp = ctx.enter_context(tc.tile_pool(name="otp", bufs=1))
    fin = ctx.enter_context(tc.tile_pool(name="fin", bufs=2))

    p_gen = {}  # pair -> [sub][k] tile handles
    ot_sb = [None] * H

    def emit_s_step(c, k):
        """S + exp + mask-mul for pair c, key chunk k."""
        if k == 0:
            p_gen[c] = [[None] * NT for _ in range(2)]
        p_tiles = p_gen[c]
        sps = [pss.tile([P, N], F32, tag="s", name=f"s{sub}") for sub in range(2)]
        # adjacent different-row-group matmuls for tile concurrency
        for qh in range(2):
            for sub in range(2):
                r0 = sub * HD
                nc.tensor.matmul(
                    sps[sub][:, qh * NH:(qh + 1) * NH],
                    lhsT=kt[c][r0:r0 + HD, k * P:(k + 1) * P],
                    rhs=qt[c][r0:r0 + HD, qh * NH:(qh + 1) * NH],
                    start=True,
                    stop=True,
                    tile_position=(r0, 0),
                )
        for sub in range(2):
            e = etq.tile([P, N], BF16, tag="et", name="et")
            nc.scalar.activation(e, sps[sub], AF.Exp, scale=0.125)
            p_tiles[sub][k] = pp.tile(
                [P, N], BF16, tag=f"p{sub}_{k}", name=f"p{sub}_{k}",
                bufs=2 if k < 4 else 1,
            )
            nc.vector.tensor_mul(p_tiles[sub][k], e, m_sb[k])

    def emit_pv_piece(c, piece):
        """One (sub, qh) quarter of PV for pair c: 8 accumulating matmuls."""
        sub, qh = piece // 2, piece % 2
        h = 2 * c + sub
        if qh == 0:
            ot_sb[h] = otp.tile([HD1, N], BF16, tag=f"ot{sub}", name=f"ot{h}")
        p_tiles = p_gen[c]
        ops = pso.tile([HD1, NH], F32, tag="ov", name="ov")
        for k in range(NT):
            nc.tensor.matmul(
                ops,
                lhsT=v_sb[k][:, h * HD1:(h + 1) * HD1],
                rhs=p_tiles[sub][k][:, qh * NH:(qh + 1) * NH],
                start=(k == 0),
                stop=(k == NT - 1),
            )
        nc.vector.tensor_copy(ot_sb[h][:, qh * NH:(qh + 1) * NH], ops)
        if piece == 3:
            del p_gen[c]

    def emit_fin_qc(c, qc):
        # transpose the pair's out^T for q-chunk qc, scale by 1/rowsum, DMA out
        fp = work.tile([P, NH], BF16, tag="work", name="fp")
        for sub in range(2):
            # stride 66 keeps the bf16 psum write 4-byte aligned
            nc.tensor.transpose(
                fp[:, sub * 66:sub * 66 + HD1],
                ot_sb[2 * c + sub][:, qc * P:(qc + 1) * P],
                identb[0:HD1, 0:HD1],
            )
        fp3 = fp[:, 0:132].rearrange("p (h j) -> p h j", j=66)
        rec = fin.tile([P, 2], F32, tag="rec", name="rec")
        nc.vector.reciprocal(rec, fp3[:, :, HD:HD1].squeeze(-1))
        rec_b = bass.AP(
            tensor=rec.tensor, offset=rec.offset, ap=list(rec.ap) + [[0, HD]]
        )
        out3 = out_sb[qc].rearrange("p (h j) -> p h j", j=HD)
        nc.vector.tensor_mul(out3[:, 2 * c:2 * c + 2, :], fp3[:, :, 0:HD], rec_b)
        if c in (2, 4, 5):
            lo, hi = {2: (0, 384), 4: (384, 640), 5: (640, 768)}[c]
            nc.sync.dma_start(
                out=out_d[qc * P:(qc + 1) * P, lo:hi],
                in_=out_sb[qc][:, lo:hi],
            )

    # Last-pair PV chains for q-half 0, fed incrementally during block 5's
    # fin steps so only the q-half-1 chains remain for the drain.
    last_chains = {}

    def open_last_chains(cc):
        for sub in range(2):
            ot_sb[2 * cc + sub] = otp.tile(
                [HD1, N], BF16, tag=f"ot{sub}", name=f"ot{2 * cc + sub}"
            )
            last_chains[sub] = pso.tile([HD1, NH], F32, tag="ov", name=f"lc{sub}")

    def feed_last_chains(cc, ks):
        p_tiles = p_gen[cc]
        for sub in range(2):
            h = 2 * cc + sub
            for kk in ks:
                nc.tensor.matmul(
                    last_chains[sub],
                    lhsT=v_sb[kk][:, h * HD1:(h + 1) * HD1],
                    rhs=p_tiles[sub][kk][:, 0:NH],
                    start=(kk == 0),
                    stop=(kk == NT - 1),
                )

    def close_last_chains(cc):
        for sub in range(2):
            nc.vector.tensor_copy(ot_sb[2 * cc + sub][:, 0:NH], last_chains[sub])

    # Steady-state weave per block c:
    #   k step:  S(c,k), then
    #     c==0:  masks k+2, adj dma k+4, V chunk k
    #     c>=1:  k<4: PV piece k of pair c-1;  k>=4: finalize q-chunks of c-1
    #            plus one QK(c+1) projection unit at odd k
    for c in range(NPAIR):
        for k in range(NT):
            if c == 0:
                emit_mask(k)
            emit_s_step(c, k)
            if c == 0:
                if k >= 3:
                    emit_v(k - 3)
                if k == NT - 1:
                    for i in range(NT - 3, NT):
                        emit_v(i)
            else:
                if k < 4:
                    emit_pv_piece(c - 1, k)
                else:
                    emit_fin_qc(c - 1, 2 * (k - 4))
                    emit_fin_qc(c - 1, 2 * (k - 4) + 1)
                if k >= 4 and c + 1 < DT:
                    wname, dst, bias_sb = (
                        ("wq", qt, bq_sb) if k < 6 else ("wk", kt, bk_sb)
                    )
                    emit_qk_chunk(c + 1, wname, dst, bias_sb, k % 2)
                if c == NPAIR - 1 and k >= 4:
                    if k == 4:
                        open_last_chains(c)
                        feed_last_chains(c, range(4))
                    else:
                        feed_last_chains(c, [k - 1])
    # drain: q-half-0 fins (qc 0-3) only need the incrementally-fed chains,
    # so they overlap the q-half-1 PV pieces on the PE queue
    feed_last_chains(NPAIR - 1, [NT - 1])
    close_last_chains(NPAIR - 1)
    emit_pv_piece(NPAIR - 1, 1)
    emit_fin_qc(NPAIR - 1, 0)
    emit_fin_qc(NPAIR - 1, 1)
    emit_pv_piece(NPAIR - 1, 3)
    emit_fin_qc(NPAIR - 1, 2)
    emit_fin_qc(NPAIR - 1, 3)
    for qc in range(4, NT):
        emit_fin_qc(NPAIR - 1, qc)


def build_nc():
    nc = bacc.Bacc("TRN2", target_bir_lowering=False, debug=False, num_devices=B)
    x_d = nc.dram_tensor("x", [N, D], BF16, kind="ExternalInput").ap()
    adj_d = nc.dram_tensor("adj", [N, N], BF16, kind="ExternalInput").ap()
    w_d = {
        "wq": nc.dram_tensor("wq", [D, D], BF16, kind="ExternalInput").ap(),
        "wk": nc.dram_tensor("wk", [D, D], BF16, kind="ExternalInput").ap(),
        "wv": nc.dram_tensor("wv", [D, D], BF16, kind="ExternalInput").ap(),
    }
    b_d = {
        "bq": nc.dram_tensor("bq", [D], F32, kind="ExternalInput").ap(),
        "bk": nc.dram_tensor("bk", [D], F32, kind="ExternalInput").ap(),
        "bv": nc.dram_tensor("bv", [D], F32, kind="ExternalInput").ap(),
    }
    beta_d = nc.dram_tensor("beta", [1, 1], F32, kind="ExternalInput").ap()
    out_d = nc.dram_tensor("out", [N, D], BF16, kind="ExternalOutput").ap()
    with tile.TileContext(nc) as tc, ExitStack() as ctx:
        _emit(tc, ctx, x_d, adj_d, w_d, b_d, beta_d, out_d)
    nc.compile()
    return nc


_CACHE = {}


def _get_nc():
    if "nc" not in _CACHE:
        _CACHE["nc"] = build_nc()
    return _CACHE["nc"]


def make_in_maps(input_graph, adj, Wq, bq, Wk, bk, Wv, bv, beta):
    import ml_dtypes

    f = lambda a: np.ascontiguousarray(np.asarray(a), dtype=np.float32)
    h = lambda a: np.ascontiguousarray(np.asarray(a, dtype=np.float32)).astype(
        ml_dtypes.bfloat16
    )
    wq, wk, wv = h(Wq), h(Wk), h(Wv)
    bqa, bka, bva = f(bq), f(bk), f(bv)
    beta_a = f(beta).reshape(1, 1)
    ig, ad = h(input_graph), h(adj)
    return [
        {
            "x": ig[b], "adj": ad[b],
            "wq": wq, "wk": wk, "wv": wv,
            "bq": bqa, "bk": bka, "bv": bva,
            "beta": beta_a,
        }
        for b in range(B)
    ]


def run_hw(in_maps, **kwargs):
    nc = _get_nc()
    return run_bass_kernel_spmd(nc, in_maps, list(range(B)), **kwargs)


def kernel(input_graph, adj, Wq, bq, Wk, bk, Wv, bv, beta):
    in_maps = make_in_maps(input_graph, adj, Wq, bq, Wk, bk, Wv, bv, beta)
    res = run_hw(in_maps)
    return np.stack(
        [np.asarray(res.results[i]["out"], dtype=np.float32) for i in range(B)], axis=0
    )


# revision 41
# speedup vs baseline: 1.1980x; 1.0045x over previous
"""Masked multi-head attention (sparse_attention) Trainium2 Bass kernel.

Data-parallel over batch: B=8 batch elements, one per NeuronCore.
Per-core computation for batch element b (all shapes hardcoded):
  x [1024,768], adj [1024,1024], Wq/Wk/Wv [768,768], bq/bk/bv [768], beta []
  q = x@Wq+bq; k = x@Wk+bk; v = x@Wv+bv      (12 heads of 64)
  S = q k^T / 8 + beta*adj ; masked where adj<=0 ; P = softmax(S)
  out = P v  -> [1024, 768]

Design (v3, ~195us vs 242us phase-serial baseline):

1. Single fused software pipeline over head pairs.  The ACT engine owns
   the 12.6M-element exp (~92us at 1 elem/cycle/lane) and must run the
   whole kernel, not just a trailing attention phase.  Attention for
   pair c starts as soon as qt[c]/kt[c] exist (~t=15us); V-projection,
   late QK chunks, masks, finalize and output DMA are dribbled into the
   per-k weave so PE/DVE/DMA stay busy under the ACT/PE roofline.
   Weave per block c: step k emits S(c,k)+exp+mask-mul, then one PV
   quarter of pair c-1 (k<4) or two finalize q-chunks of pair c-1 plus
   one QK(c+1) projection unit (k>=4).  The last pair's q-half-0 PV
   chains are fed incrementally during block 5 and its fins interleave
   the q-half-1 pieces, shortening the pipeline drain.

2. bf16 inputs, cast on host.  The kernel computes in bf16 anyway;
   shipping x/adj/W as bf16 halves input HBM traffic (13.75 -> 6.9 MB,
   the startup wall is DMA-bandwidth-bound) and removes all SWDGE
   casting DMAs.  DMA delivery order matches consumption: x, Wq, Wk,
   adj half-0 (masks 0-3), Wv, adj half-1.  Output is bf16 too, cast
   back on host; rel err ~6.4e-3 (vs 5.7e-3 all-f32-io).

3. Per-core dataflow (per head pair c = heads 2c, 2c+1):
   X^T via PE transposes (bf16, psum) evacuated to SBUF by ACT/DVE;
   Q^T,K^T,V = bf16 matmuls.  m^T[k] = (adjT>0)*exp(beta*adjT) via PE
   transposes of adj column slices.  S^T pair = K_h @ Q_h^T as two
   row-tiled K=64 matmuls (tile_position row groups, emitted adjacent;
   they launch concurrently when ACT has freed both psum slots).
   P^T = m^T * exp(S^T/8) (ACT exp + DVE mask-mul, both bf16).
   out^T[65,512] = [V_h|1]^T @ P^T -- the ones column emits softmax
   row-sums for free.  PE-transpose back per q-chunk, scale by
   1/rowsum, DMA out in thirds (after pairs 2, 4, 5).

   PSUM: 2 banks transient work ring (proj/masks/finalize), 4 banks
   S ring (2 x [128,1024]), 2 banks PV accumulators.  P tiles for
   k<4 are double-buffered so pair c+1's mask-muls decouple from
   pair c's PV reads (deadlock- and stall-avoidance in the weave).

Rejected by measurement: fp8 S/PV (logit error ~5% > 2e-2 budget),
PV column-packing with M=1 rowsum chains (col tiling disables fast
weight load; +46us), adj column-strip DMA (512B-descriptor storm on
the sync queue), [128,512] S tiles with 4-5 deep rings (+10us ACT
overhead beats the concurrency gain), full P double-buffering (+34us,
scheduler pathology).
"""

import sys

import numpy as np

try:
    import concourse.bass as bass
except ImportError:  # container default location
    sys.path.insert(0, "/opt/trn_rl_repo")
    import concourse.bass as bass

from contextlib import ExitStack

import concourse.bacc as bacc
import concourse.mybir as mybir
import concourse.tile as tile
from concourse.bass_utils import run_bass_kernel_spmd
from concourse.masks import make_identity

B, N, D, H = 8, 1024, 768, 12
HD = 64
P = 128
NT = N // P  # 8 row chunks
DT = D // P  # 6 feature chunks
NH = 512  # free-dim tile for matmuls
HD1 = HD + 1  # head dim + ones column
NPAIR = H // 2  # 6 head pairs

F32 = mybir.dt.float32
BF16 = mybir.dt.bfloat16
AF = mybir.ActivationFunctionType
ALU = mybir.AluOpType


def _emit(tc, ctx, x_d, adj_d, w_d, b_d, beta_d, out_d):
    nc = tc.nc

    const = ctx.enter_context(tc.tile_pool(name="const", bufs=1))
    ident = const.tile([P, P], F32, tag="ident")
    make_identity(nc, ident)
    identb = const.tile([P, P], BF16, tag="identb")
    make_identity(nc, identb)

    def bcast(ap, n_part):
        return bass.AP(tensor=ap.tensor, offset=ap.offset, ap=[[0, n_part]] + list(ap.ap))

    beta_sb = const.tile([P, 1], F32, tag="beta")
    nc.gpsimd.dma_start(out=beta_sb, in_=bcast(beta_d[0], P))
    bq_sb = const.tile([P, DT], F32, tag="bq")
    bk_sb = const.tile([P, DT], F32, tag="bk")
    bv_bc = const.tile([P, D], F32, tag="bv")

    # ---- persistent tensors ----
    pers = ctx.enter_context(tc.tile_pool(name="pers", bufs=1))
    w_sb = {}
    for wname in ("wq", "wk", "wv"):
        w_sb[wname] = [
            pers.tile([P, D], BF16, tag=f"{wname}{c}", name=f"{wname}{c}")
            for c in range(DT)
        ]
    xt = [pers.tile([P, N], BF16, tag=f"xt{c}", name=f"xt{c}") for c in range(DT)]
    qt = [pers.tile([P, N], BF16, tag=f"qt{c}", name=f"qt{c}") for c in range(DT)]
    kt = [pers.tile([P, N], BF16, tag=f"kt{c}", name=f"kt{c}") for c in range(DT)]
    v_sb = [pers.tile([P, H * HD1], BF16, tag=f"v{i}", name=f"v{i}") for i in range(NT)]
    m_sb = [pers.tile([P, N], BF16, tag=f"m{k}", name=f"m{k}") for k in range(NT)]
    out_sb = [pers.tile([P, D], BF16, tag=f"os{i}", name=f"os{i}") for i in range(NT)]

    # ---- input DMAs (issue everything up front; engines overlap) ----
    # adjp outlives xp: create it first so closing xp pops the alloc stack.
    adjp = ctx.enter_context(tc.tile_pool(name="adjp", bufs=1))
    es_x = ExitStack()
    xp = es_x.enter_context(tc.tile_pool(name="xp", bufs=1))
    x_sb = [xp.tile([P, D], BF16, tag=f"x{i}", name=f"x{i}") for i in range(NT)]
    for i in range(NT):
        nc.sync.dma_start(out=x_sb[i], in_=x_d[i * P:(i + 1) * P, :])

    # biases on the (otherwise idle) gpsimd queue
    nc.gpsimd.dma_start(out=bq_sb, in_=b_d["bq"].rearrange("(c p) -> p c", p=P))
    nc.gpsimd.dma_start(out=bk_sb, in_=b_d["bk"].rearrange("(c p) -> p c", p=P))
    nc.gpsimd.dma_start(out=bv_bc, in_=bcast(b_d["bv"], P))

    # adj in column-half tiles: masks k<4 only gate on the first half.
    adj_sb = [
        [adjp.tile([P, NH], BF16, tag=f"aj{j}_{h}", name=f"aj{j}_{h}") for h in range(2)]
        for j in range(NT)
    ]
    # delivery order: wq, wk (gate S(0)), adj half 0 (masks 0-3), wv, adj half 1
    for wname in ("wq", "wk"):
        for c in range(DT):
            nc.sync.dma_start(
                out=w_sb[wname][c], in_=w_d[wname][c * P:(c + 1) * P, :]
            )
    for j in range(NT):
        nc.sync.dma_start(out=adj_sb[j][0], in_=adj_d[j * P:(j + 1) * P, 0:NH])
    for c in range(DT):
        nc.sync.dma_start(out=w_sb["wv"][c], in_=w_d["wv"][c * P:(c + 1) * P, :])
    for j in range(NT):
        nc.sync.dma_start(out=adj_sb[j][1], in_=adj_d[j * P:(j + 1) * P, NH:N])

    # ---- psum pools ----
    work = ctx.enter_context(tc.tile_pool(name="work", space="PSUM", bufs=2))
    pss = ctx.enter_context(tc.tile_pool(name="pss", space="PSUM", bufs=2))
    pso = ctx.enter_context(tc.tile_pool(name="pso", space="PSUM", bufs=2))

    # ---- X^T: per c, two groups of 4 PE transposes + one evac ----
    # g=0 groups first: they only need x tiles 0-3, so PE starts sooner.
    # Evacs split ACT/DVE to relieve the ACT pileup around block 0.
    for g in range(2):
        for c in range(DT):
            tp = work.tile([P, NH], BF16, tag="work", name="tp")
            for bb in range(4):
                i = g * 4 + bb
                nc.tensor.transpose(
                    tp[:, bb * P:(bb + 1) * P], x_sb[i][:, c * P:(c + 1) * P], identb
                )
            if c % 2 == g % 2:
                nc.scalar.copy(xt[c][:, g * NH:(g + 1) * NH], tp)
            else:
                nc.vector.tensor_copy(xt[c][:, g * NH:(g + 1) * NH], tp)
    es_x.close()

    # ---- masks: m^T[k] = (adjT>0)*exp(beta*adjT) ----
    etm = ctx.enter_context(tc.tile_pool(name="etm", bufs=2))

    def emit_mask(k):
        for g in range(2):
            tp = work.tile([P, NH], BF16, tag="work", name="mtp")
            for bb in range(4):
                j = g * 4 + bb
                nc.tensor.transpose(
                    tp[:, bb * P:(bb + 1) * P],
                    adj_sb[j][k // 4][:, (k % 4) * P:(k % 4 + 1) * P],
                    identb,
                )
            e_m = etm.tile([P, NH], BF16, tag="em", name="em")
            nc.scalar.activation(e_m, tp, AF.Exp, scale=beta_sb[:, 0:1])
            nc.vector.scalar_tensor_tensor(
                out=m_sb[k][:, g * NH:(g + 1) * NH],
                in0=tp,
                scalar=0.0,
                in1=e_m,
                op0=ALU.is_gt,
                op1=ALU.mult,
            )

    def emit_qk_chunk(c, wname, dst, bias_sb, qh):
        mm = work.tile([P, NH], F32, tag="work", name="mm")
        for kc in range(DT):
            nc.tensor.matmul(
                mm,
                lhsT=w_sb[wname][kc][:, c * P:(c + 1) * P],
                rhs=xt[kc][:, qh * NH:(qh + 1) * NH],
                start=(kc == 0),
                stop=(kc == DT - 1),
            )
        nc.vector.tensor_scalar_add(
            dst[c][:, qh * NH:(qh + 1) * NH], mm, bias_sb[:, c:c + 1]
        )

    def emit_v(i):
        for s, w in ((0, NH), (NH, D - NH)):
            mm = work.tile([P, NH], F32, tag="work", name="vmm")
            for kc in range(DT):
                nc.tensor.matmul(
                    mm[:, 0:w],
                    lhsT=xt[kc][:, i * P:(i + 1) * P],
                    rhs=w_sb["wv"][kc][:, s:s + w],
                    start=(kc == 0),
                    stop=(kc == DT - 1),
                )
            nh = w // HD
            h0 = s // HD
            dst3 = v_sb[i].rearrange("p (h j) -> p h j", j=HD1)[:, h0:h0 + nh, 0:HD]
            src3 = mm[:, 0:w].rearrange("p (h j) -> p h j", j=HD)
            bias3 = bv_bc[:, s:s + w].rearrange("p (h j) -> p h j", j=HD)
            nc.vector.tensor_add(dst3, src3, bias3)
        ones3 = v_sb[i].rearrange("p (h j) -> p h j", j=HD1)[:, :, HD:HD1]
        nc.vector.memset(ones3, 1.0)

    # first two QK chunks before the attention pipeline starts
    for c in (0, 1):
        for wname, dst, bias_sb in (("wq", qt, bq_sb), ("wk", kt, bk_sb)):
            for qh in range(2):
                emit_qk_chunk(c, wname, dst, bias_sb, qh)

    # ---- attention pipeline over head pairs ----
    etq = ctx.enter_context(tc.tile_pool(name="etq", bufs=10))
    pp = ctx.enter_context(tc.tile_pool(name="pp", bufs=1))
    otp = ctx.enter_context(tc.tile_pool(name="otp", bufs=1))
    fin = ctx.enter_context(tc.tile_pool(name="fin", bufs=2))

    p_gen = {}  # pair -> [sub][k] tile handles
    ot_sb = [None] * H

    def emit_s_step(c, k):
        """S + exp + mask-mul for pair c, key chunk k."""
        if k == 0:
            p_gen[c] = [[None] * NT for _ in range(2)]
        p_tiles = p_gen[c]
        sps = [pss.tile([P, N], F32, tag="s", name=f"s{sub}") for sub in range(2)]
        # adjacent different-row-group matmuls for tile concurrency
        for qh in range(2):
            for sub in range(2):
                r0 = sub * HD
                nc.tensor.matmul(
                    sps[sub][:, qh * NH:(qh + 1) * NH],
                    lhsT=kt[c][r0:r0 + HD, k * P:(k + 1) * P],
                    rhs=qt[c][r0:r0 + HD, qh * NH:(qh + 1) * NH],
                    start=True,
                    stop=True,
                    tile_position=(r0, 0),
                )
        for sub in range(2):
            e = etq.tile([P, N], BF16, tag="et", name="et")
            nc.scalar.activation(e, sps[sub], AF.Exp, scale=0.125)
            p_tiles[sub][k] = pp.tile(
                [P, N], BF16, tag=f"p{sub}_{k}", name=f"p{sub}_{k}",
                bufs=2 if k < 4 else 1,
            )
            nc.vector.tensor_mul(p_tiles[sub][k], e, m_sb[k])

    def emit_pv_piece(c, piece):
        """One (sub, qh) quarter of PV for pair c: 8 accumulating matmuls."""
        sub, qh = piece // 2, piece % 2
        h = 2 * c + sub
        if qh == 0:
            ot_sb[h] = otp.tile([HD1, N], BF16, tag=f"ot{sub}", name=f"ot{h}")
        p_tiles = p_gen[c]
        ops = pso.tile([HD1, NH], F32, tag="ov", name="ov")
        for k in range(NT):
            nc.tensor.matmul(
                ops,
                lhsT=v_sb[k][:, h * HD1:(h + 1) * HD1],
                rhs=p_tiles[sub][k][:, qh * NH:(qh + 1) * NH],
                start=(k == 0),
                stop=(k == NT - 1),
            )
        nc.vector.tensor_copy(ot_sb[h][:, qh * NH:(qh + 1) * NH], ops)
        if piece == 3:
            del p_gen[c]

    def emit_fin_qc(c, qc):
        # transpose the pair's out^T for q-chunk qc, scale by 1/rowsum, DMA out
        fp = work.tile([P, NH], BF16, tag="work", name="fp")
        for sub in range(2):
            # stride 66 keeps the bf16 psum write 4-byte aligned
            nc.tensor.transpose(
                fp[:, sub * 66:sub * 66 + HD1],
                ot_sb[2 * c + sub][:, qc * P:(qc + 1) * P],
                identb[0:HD1, 0:HD1],
            )
        fp3 = fp[:, 0:132].rearrange("p (h j) -> p h j", j=66)
        rec = fin.tile([P, 2], F32, tag="rec", name="rec")
        nc.vector.reciprocal(rec, fp3[:, :, HD:HD1].squeeze(-1))
        rec_b = bass.AP(
            tensor=rec.tensor, offset=rec.offset, ap=list(rec.ap) + [[0, HD]]
        )
        out3 = out_sb[qc].rearrange("p (h j) -> p h j", j=HD)
        nc.vector.tensor_mul(out3[:, 2 * c:2 * c + 2, :], fp3[:, :, 0:HD], rec_b)
        if c in (2, 4, 5):
            lo, hi = {2: (0, 384), 4: (384, 640), 5: (640, 768)}[c]
            nc.sync.dma_start(
                out=out_d[qc * P:(qc + 1) * P, lo:hi],
                in_=out_sb[qc][:, lo:hi],
            )

    # Last-pair PV chains for q-half 0, fed incrementally during block 5's
    # fin steps so only the q-half-1 chains remain for the drain.
    last_chains = {}

    def open_last_chains(cc):
        for sub in range(2):
            ot_sb[2 * cc + sub] = otp.tile(
                [HD1, N], BF16, tag=f"ot{sub}", name=f"ot{2 * cc + sub}"
            )
            last_chains[sub] = pso.tile([HD1, NH], F32, tag="ov", name=f"lc{sub}")

    def feed_last_chains(cc, ks):
        p_tiles = p_gen[cc]
        for sub in range(2):
            h = 2 * cc + sub
            for kk in ks:
                nc.tensor.matmul(
                    last_chains[sub],
                    lhsT=v_sb[kk][:, h * HD1:(h + 1) * HD1],
                    rhs=p_tiles[sub][kk][:, 0:NH],
                    start=(kk == 0),
                    stop=(kk == NT - 1),
                )

    def close_last_chains(cc):
        for sub in range(2):
            nc.vector.tensor_copy(ot_sb[2 * cc + sub][:, 0:NH], last_chains[sub])

    # Steady-state weave per block c:
    #   k step:  S(c,k), then
    #     c==0:  masks k+2, adj dma k+4, V chunk k
    #     c>=1:  k<4: PV piece k of pair c-1;  k>=4: finalize q-chunks of c-1
    #            plus one QK(c+1) projection unit at odd k
    for c in range(NPAIR):
        for k in range(NT):
            if c == 0:
                emit_mask(k)
            emit_s_step(c, k)
            if c == 0:
                if k >= 3:
                    emit_v(k - 3)
                if k == NT - 1:
                    for i in range(NT - 3, NT):
                        emit_v(i)
            else:
                if k < 4:
                    emit_pv_piece(c - 1, k)
                else:
                    emit_fin_qc(c - 1, 2 * (k - 4))
                    emit_fin_qc(c - 1, 2 * (k - 4) + 1)
                if k >= 4 and c + 1 < DT:
                    wname, dst, bias_sb = (
                        ("wq", qt, bq_sb) if k < 6 else ("wk", kt, bk_sb)
                    )
                    emit_qk_chunk(c + 1, wname, dst, bias_sb, k % 2)
                if c == NPAIR - 1 and k >= 4:
                    if k == 4:
                        open_last_chains(c)
                        feed_last_chains(c, range(4))
                    else:
                        feed_last_chains(c, [k - 1])
    # drain: q-half-0 fins (qc 0-3) only need the incrementally-fed chains,
    # so they overlap the q-half-1 PV pieces on the PE queue
    feed_last_chains(NPAIR - 1, [NT - 1])
    close_last_chains(NPAIR - 1)
    emit_pv_piece(NPAIR - 1, 1)
    emit_fin_qc(NPAIR - 1, 0)
    emit_fin_qc(NPAIR - 1, 1)
    emit_pv_piece(NPAIR - 1, 3)
    emit_fin_qc(NPAIR - 1, 2)
    emit_fin_qc(NPAIR - 1, 3)
    for qc in range(4, NT):
        emit_fin_qc(NPAIR - 1, qc)


def build_nc():
    nc = bacc.Bacc("TRN2", target_bir_lowering=False, debug=False, num_devices=B)
    x_d = nc.dram_tensor("x", [N, D], BF16, kind="ExternalInput").ap()
    adj_d = nc.dram_tensor("adj", [N, N], BF16, kind="ExternalInput").ap()
    w_d = {
        "wq": nc.dram_tensor("wq", [D, D], BF16, kind="ExternalInput").ap(),
        "wk": nc.dram_tensor("wk", [D, D], BF16, kind="ExternalInput").ap(),
        "wv": nc.dram_tensor("wv", [D, D], BF16, kind="ExternalInput").ap(),
    }
    b_d = {
        "bq": nc.dram_tensor("bq", [D], F32, kind="ExternalInput").ap(),
        "bk": nc.dram_tensor("bk", [D], F32, kind="ExternalInput").ap(),
        "bv": nc.dram_tensor("bv", [D], F32, kind="ExternalInput").ap(),
    }
    beta_d = nc.dram_tensor("beta", [1, 1], F32, kind="ExternalInput").ap()
    out_d = nc.dram_tensor("out", [N, D], BF16, kind="ExternalOutput").ap()
    with tile.TileContext(nc) as tc, ExitStack() as ctx:
        _emit(tc, ctx, x_d, adj_d, w_d, b_d, beta_d, out_d)
    nc.compile()
    return nc


_CACHE = {}


def _get_nc():
    if "nc" not in _CACHE:
        _CACHE["nc"] = build_nc()
    return _CACHE["nc"]


def make_in_maps(input_graph, adj, Wq, bq, Wk, bk, Wv, bv, beta):
    import ml_dtypes

    f = lambda a: np.ascontiguousarray(np.asarray(a), dtype=np.float32)
    h = lambda a: np.ascontiguousarray(np.asarray(a, dtype=np.float32)).astype(
        ml_dtypes.bfloat16
    )
    wq, wk, wv = h(Wq), h(Wk), h(Wv)
    bqa, bka, bva = f(bq), f(bk), f(bv)
    beta_a = f(beta).reshape(1, 1)
    ig, ad = h(input_graph), h(adj)
    return [
        {
            "x": ig[b], "adj": ad[b],
            "wq": wq, "wk": wk, "wv": wv,
            "bq": bqa, "bk": bka, "bv": bva,
            "beta": beta_a,
        }
        for b in range(B)
    ]


def run_hw(in_maps, **kwargs):
    nc = _get_nc()
    return run_bass_kernel_spmd(nc, in_maps, list(range(B)), **kwargs)


def kernel(input_graph, adj, Wq, bq, Wk, bk, Wv, bv, beta):
    in_maps = make_in_maps(input_graph, adj, Wq, bq, Wk, bk, Wv, bv, beta)
    res = run_hw(in_maps)
    return np.stack(
        [np.asarray(res.results[i]["out"], dtype=np.float32) for i in range(B)], axis=0
    )


# revision 42
# speedup vs baseline: 1.1988x; 1.0006x over previous
"""Masked multi-head attention (sparse_attention) Trainium2 Bass kernel.

Data-parallel over batch: B=8 batch elements, one per NeuronCore.
Per-core computation for batch element b (all shapes hardcoded):
  x [1024,768], adj [1024,1024], Wq/Wk/Wv [768,768], bq/bk/bv [768], beta []
  q = x@Wq+bq; k = x@Wk+bk; v = x@Wv+bv      (12 heads of 64)
  S = q k^T / 8 + beta*adj ; masked where adj<=0 ; P = softmax(S)
  out = P v  -> [1024, 768]

Design (v3, ~195us vs 242us phase-serial baseline):

1. Single fused software pipeline over head pairs.  The ACT engine owns
   the 12.6M-element exp (~92us at 1 elem/cycle/lane) and must run the
   whole kernel, not just a trailing attention phase.  Attention for
   pair c starts as soon as qt[c]/kt[c] exist (~t=15us); V-projection,
   late QK chunks, masks, finalize and output DMA are dribbled into the
   per-k weave so PE/DVE/DMA stay busy under the ACT/PE roofline.
   Weave per block c: step k emits S(c,k)+exp+mask-mul, then one PV
   quarter of pair c-1 (k<4) or two finalize q-chunks of pair c-1 plus
   one QK(c+1) projection unit (k>=4).  The last pair's q-half-0 PV
   chains are fed incrementally during block 5 and its fins interleave
   the q-half-1 pieces, shortening the pipeline drain.

2. bf16 inputs, cast on host.  The kernel computes in bf16 anyway;
   shipping x/adj/W as bf16 halves input HBM traffic (13.75 -> 6.9 MB,
   the startup wall is DMA-bandwidth-bound) and removes all SWDGE
   casting DMAs.  DMA delivery order matches consumption: x, Wq, Wk,
   adj half-0 (masks 0-3), Wv, adj half-1.  Output is bf16 too, cast
   back on host; rel err ~6.4e-3 (vs 5.7e-3 all-f32-io).

3. Per-core dataflow (per head pair c = heads 2c, 2c+1):
   X^T via PE transposes (bf16, psum) evacuated to SBUF by ACT/DVE;
   Q^T,K^T,V = bf16 matmuls.  m^T[k] = (adjT>0)*exp(beta*adjT) via PE
   transposes of adj column slices.  S^T pair = K_h @ Q_h^T as two
   row-tiled K=64 matmuls (tile_position row groups, emitted adjacent;
   they launch concurrently when ACT has freed both psum slots).
   P^T = m^T * exp(S^T/8) (ACT exp + DVE mask-mul, both bf16).
   out^T[65,512] = [V_h|1]^T @ P^T -- the ones column emits softmax
   row-sums for free.  PE-transpose back per q-chunk, scale by
   1/rowsum, DMA out in thirds (after pairs 2, 4, 5).

   PSUM: 2 banks transient work ring (proj/masks/finalize), 4 banks
   S ring (2 x [128,1024]), 2 banks PV accumulators.  P tiles for
   k<4 are double-buffered so pair c+1's mask-muls decouple from
   pair c's PV reads (deadlock- and stall-avoidance in the weave).

Rejected by measurement: fp8 S/PV (logit error ~5% > 2e-2 budget),
PV column-packing with M=1 rowsum chains (col tiling disables fast
weight load; +46us), adj column-strip DMA (512B-descriptor storm on
the sync queue), [128,512] S tiles with 4-5 deep rings (+10us ACT
overhead beats the concurrency gain), full P double-buffering (+34us,
scheduler pathology).
"""

import sys

import numpy as np

try:
    import concourse.bass as bass
except ImportError:  # container default location
    sys.path.insert(0, "/opt/trn_rl_repo")
    import concourse.bass as bass

from contextlib import ExitStack

import concourse.bacc as bacc
import concourse.mybir as mybir
import concourse.tile as tile
from concourse.bass_utils import run_bass_kernel_spmd
from concourse.masks import make_identity

B, N, D, H = 8, 1024, 768, 12
HD = 64
P = 128
NT = N // P  # 8 row chunks
DT = D // P  # 6 feature chunks
NH = 512  # free-dim tile for matmuls
HD1 = HD + 1  # head dim + ones column
NPAIR = H // 2  # 6 head pairs

F32 = mybir.dt.float32
BF16 = mybir.dt.bfloat16
AF = mybir.ActivationFunctionType
ALU = mybir.AluOpType


def _emit(tc, ctx, x_d, adj_d, w_d, b_d, beta_d, out_d):
    nc = tc.nc

    const = ctx.enter_context(tc.tile_pool(name="const", bufs=1))
    ident = const.tile([P, P], F32, tag="ident")
    make_identity(nc, ident)
    identb = const.tile([P, P], BF16, tag="identb")
    make_identity(nc, identb)

    def bcast(ap, n_part):
        return bass.AP(tensor=ap.tensor, offset=ap.offset, ap=[[0, n_part]] + list(ap.ap))

    beta_sb = const.tile([P, 1], F32, tag="beta")
    nc.gpsimd.dma_start(out=beta_sb, in_=bcast(beta_d[0], P))
    bq_sb = const.tile([P, DT], F32, tag="bq")
    bk_sb = const.tile([P, DT], F32, tag="bk")
    bv_bc = const.tile([P, D], F32, tag="bv")

    # ---- persistent tensors ----
    pers = ctx.enter_context(tc.tile_pool(name="pers", bufs=1))
    w_sb = {}
    for wname in ("wq", "wk", "wv"):
        w_sb[wname] = [
            pers.tile([P, D], BF16, tag=f"{wname}{c}", name=f"{wname}{c}")
            for c in range(DT)
        ]
    xt = [pers.tile([P, N], BF16, tag=f"xt{c}", name=f"xt{c}") for c in range(DT)]
    qt = [pers.tile([P, N], BF16, tag=f"qt{c}", name=f"qt{c}") for c in range(DT)]
    kt = [pers.tile([P, N], BF16, tag=f"kt{c}", name=f"kt{c}") for c in range(DT)]
    v_sb = [pers.tile([P, H * HD1], BF16, tag=f"v{i}", name=f"v{i}") for i in range(NT)]
    m_sb = [pers.tile([P, N], BF16, tag=f"m{k}", name=f"m{k}") for k in range(NT)]
    out_sb = [pers.tile([P, D], BF16, tag=f"os{i}", name=f"os{i}") for i in range(NT)]

    # ---- input DMAs (issue everything up front; engines overlap) ----
    # adjp outlives xp: create it first so closing xp pops the alloc stack.
    adjp = ctx.enter_context(tc.tile_pool(name="adjp", bufs=1))
    es_x = ExitStack()
    xp = es_x.enter_context(tc.tile_pool(name="xp", bufs=1))
    x_sb = [xp.tile([P, D], BF16, tag=f"x{i}", name=f"x{i}") for i in range(NT)]
    for i in range(NT):
        nc.sync.dma_start(out=x_sb[i], in_=x_d[i * P:(i + 1) * P, :])

    # biases on the (otherwise idle) gpsimd queue
    nc.gpsimd.dma_start(out=bq_sb, in_=b_d["bq"].rearrange("(c p) -> p c", p=P))
    nc.gpsimd.dma_start(out=bk_sb, in_=b_d["bk"].rearrange("(c p) -> p c", p=P))
    nc.gpsimd.dma_start(out=bv_bc, in_=bcast(b_d["bv"], P))

    # adj in column-half tiles: masks k<4 only gate on the first half.
    adj_sb = [
        [adjp.tile([P, NH], BF16, tag=f"aj{j}_{h}", name=f"aj{j}_{h}") for h in range(2)]
        for j in range(NT)
    ]
    # delivery order: wq, wk (gate S(0)), adj half 0 (masks 0-3), wv, adj half 1
    for wname in ("wq", "wk"):
        for c in range(DT):
            nc.sync.dma_start(
                out=w_sb[wname][c], in_=w_d[wname][c * P:(c + 1) * P, :]
            )
    for j in range(NT):
        nc.sync.dma_start(out=adj_sb[j][0], in_=adj_d[j * P:(j + 1) * P, 0:NH])
    for c in range(DT):
        nc.sync.dma_start(out=w_sb["wv"][c], in_=w_d["wv"][c * P:(c + 1) * P, :])
    for j in range(NT):
        nc.sync.dma_start(out=adj_sb[j][1], in_=adj_d[j * P:(j + 1) * P, NH:N])

    # ---- psum pools ----
    work = ctx.enter_context(tc.tile_pool(name="work", space="PSUM", bufs=2))
    pss = ctx.enter_context(tc.tile_pool(name="pss", space="PSUM", bufs=2))
    pso = ctx.enter_context(tc.tile_pool(name="pso", space="PSUM", bufs=2))

    # ---- X^T: per c, two groups of 4 PE transposes + one evac ----
    # g=0 groups first: they only need x tiles 0-3, so PE starts sooner.
    # Evacs split ACT/DVE to relieve the ACT pileup around block 0.
    for g in range(2):
        for c in range(DT):
            tp = work.tile([P, NH], BF16, tag="work", name="tp")
            for bb in range(4):
                i = g * 4 + bb
                nc.tensor.transpose(
                    tp[:, bb * P:(bb + 1) * P], x_sb[i][:, c * P:(c + 1) * P], identb
                )
            if c % 2 == g % 2:
                nc.scalar.copy(xt[c][:, g * NH:(g + 1) * NH], tp)
            else:
                nc.vector.tensor_copy(xt[c][:, g * NH:(g + 1) * NH], tp)
    es_x.close()

    # ---- masks: m^T[k] = (adjT>0)*exp(beta*adjT) ----
    etm = ctx.enter_context(tc.tile_pool(name="etm", bufs=2))

    def emit_mask(k):
        for g in range(2):
            tp = work.tile([P, NH], BF16, tag="work", name="mtp")
            for bb in range(4):
                j = g * 4 + bb
                nc.tensor.transpose(
                    tp[:, bb * P:(bb + 1) * P],
                    adj_sb[j][k // 4][:, (k % 4) * P:(k % 4 + 1) * P],
                    identb,
                )
            e_m = etm.tile([P, NH], BF16, tag="em", name="em")
            nc.scalar.activation(e_m, tp, AF.Exp, scale=beta_sb[:, 0:1])
            nc.vector.scalar_tensor_tensor(
                out=m_sb[k][:, g * NH:(g + 1) * NH],
                in0=tp,
                scalar=0.0,
                in1=e_m,
                op0=ALU.is_gt,
                op1=ALU.mult,
            )

    def emit_qk_chunk(c, wname, dst, bias_sb, qh):
        mm = work.tile([P, NH], F32, tag="work", name="mm")
        for kc in range(DT):
            nc.tensor.matmul(
                mm,
                lhsT=w_sb[wname][kc][:, c * P:(c + 1) * P],
                rhs=xt[kc][:, qh * NH:(qh + 1) * NH],
                start=(kc == 0),
                stop=(kc == DT - 1),
            )
        nc.vector.tensor_scalar_add(
            dst[c][:, qh * NH:(qh + 1) * NH], mm, bias_sb[:, c:c + 1]
        )

    def emit_v(i):
        for s, w in ((0, NH), (NH, D - NH)):
            mm = work.tile([P, NH], F32, tag="work", name="vmm")
            for kc in range(DT):
                nc.tensor.matmul(
                    mm[:, 0:w],
                    lhsT=xt[kc][:, i * P:(i + 1) * P],
                    rhs=w_sb["wv"][kc][:, s:s + w],
                    start=(kc == 0),
                    stop=(kc == DT - 1),
                )
            nh = w // HD
            h0 = s // HD
            dst3 = v_sb[i].rearrange("p (h j) -> p h j", j=HD1)[:, h0:h0 + nh, 0:HD]
            src3 = mm[:, 0:w].rearrange("p (h j) -> p h j", j=HD)
            bias3 = bv_bc[:, s:s + w].rearrange("p (h j) -> p h j", j=HD)
            nc.vector.tensor_add(dst3, src3, bias3)
        ones3 = v_sb[i].rearrange("p (h j) -> p h j", j=HD1)[:, :, HD:HD1]
        nc.vector.memset(ones3, 1.0)

    # first two QK chunks before the attention pipeline starts
    for c in (0, 1):
        for wname, dst, bias_sb in (("wq", qt, bq_sb), ("wk", kt, bk_sb)):
            for qh in range(2):
                emit_qk_chunk(c, wname, dst, bias_sb, qh)

    # ---- attention pipeline over head pairs ----
    etq = ctx.enter_context(tc.tile_pool(name="etq", bufs=8))
    pp = ctx.enter_context(tc.tile_pool(name="pp", bufs=1))
    otp = ctx.enter_context(tc.tile_pool(name="otp", bufs=1))
    fin = ctx.enter_context(tc.tile_pool(name="fin", bufs=2))

    p_gen = {}  # pair -> [sub][k] tile handles
    ot_sb = [None] * H

    def emit_s_step(c, k):
        """S + exp + mask-mul for pair c, key chunk k."""
        if k == 0:
            p_gen[c] = [[None] * NT for _ in range(2)]
        p_tiles = p_gen[c]
        sps = [pss.tile([P, N], F32, tag="s", name=f"s{sub}") for sub in range(2)]
        # adjacent different-row-group matmuls for tile concurrency
        for qh in range(2):
            for sub in range(2):
                r0 = sub * HD
                nc.tensor.matmul(
                    sps[sub][:, qh * NH:(qh + 1) * NH],
                    lhsT=kt[c][r0:r0 + HD, k * P:(k + 1) * P],
                    rhs=qt[c][r0:r0 + HD, qh * NH:(qh + 1) * NH],
                    start=True,
                    stop=True,
                    tile_position=(r0, 0),
                )
        for sub in range(2):
            e = etq.tile([P, N], BF16, tag="et", name="et")
            nc.scalar.activation(e, sps[sub], AF.Exp, scale=0.125)
            p_tiles[sub][k] = pp.tile(
                [P, N], BF16, tag=f"p{sub}_{k}", name=f"p{sub}_{k}",
                bufs=2 if k < 4 else 1,
            )
            nc.vector.tensor_mul(p_tiles[sub][k], e, m_sb[k])

    def emit_pv_piece(c, piece):
        """One (sub, qh) quarter of PV for pair c: 8 accumulating matmuls."""
        sub, qh = piece // 2, piece % 2
        h = 2 * c + sub
        if qh == 0:
            ot_sb[h] = otp.tile([HD1, N], BF16, tag=f"ot{sub}", name=f"ot{h}")
        p_tiles = p_gen[c]
        ops = pso.tile([HD1, NH], F32, tag="ov", name="ov")
        for k in range(NT):
            nc.tensor.matmul(
                ops,
                lhsT=v_sb[k][:, h * HD1:(h + 1) * HD1],
                rhs=p_tiles[sub][k][:, qh * NH:(qh + 1) * NH],
                start=(k == 0),
                stop=(k == NT - 1),
            )
        nc.vector.tensor_copy(ot_sb[h][:, qh * NH:(qh + 1) * NH], ops)
        if piece == 3:
            del p_gen[c]

    def emit_fin_qc(c, qc):
        # transpose the pair's out^T for q-chunk qc, scale by 1/rowsum, DMA out
        fp = work.tile([P, NH], BF16, tag="work", name="fp")
        for sub in range(2):
            # stride 66 keeps the bf16 psum write 4-byte aligned
            nc.tensor.transpose(
                fp[:, sub * 66:sub * 66 + HD1],
                ot_sb[2 * c + sub][:, qc * P:(qc + 1) * P],
                identb[0:HD1, 0:HD1],
            )
        fp3 = fp[:, 0:132].rearrange("p (h j) -> p h j", j=66)
        rec = fin.tile([P, 2], F32, tag="rec", name="rec")
        nc.vector.reciprocal(rec, fp3[:, :, HD:HD1].squeeze(-1))
        rec_b = bass.AP(
            tensor=rec.tensor, offset=rec.offset, ap=list(rec.ap) + [[0, HD]]
        )
        out3 = out_sb[qc].rearrange("p (h j) -> p h j", j=HD)
        nc.vector.tensor_mul(out3[:, 2 * c:2 * c + 2, :], fp3[:, :, 0:HD], rec_b)
        if c in (2, 4, 5):
            lo, hi = {2: (0, 384), 4: (384, 640), 5: (640, 768)}[c]
            nc.sync.dma_start(
                out=out_d[qc * P:(qc + 1) * P, lo:hi],
                in_=out_sb[qc][:, lo:hi],
            )

    # Last-pair PV chains for q-half 0, fed incrementally during block 5's
    # fin steps so only the q-half-1 chains remain for the drain.
    last_chains = {}

    def open_last_chains(cc):
        for sub in range(2):
            ot_sb[2 * cc + sub] = otp.tile(
                [HD1, N], BF16, tag=f"ot{sub}", name=f"ot{2 * cc + sub}"
            )
            last_chains[sub] = pso.tile([HD1, NH], F32, tag="ov", name=f"lc{sub}")

    def feed_last_chains(cc, ks):
        p_tiles = p_gen[cc]
        for sub in range(2):
            h = 2 * cc + sub
            for kk in ks:
                nc.tensor.matmul(
                    last_chains[sub],
                    lhsT=v_sb[kk][:, h * HD1:(h + 1) * HD1],
                    rhs=p_tiles[sub][kk][:, 0:NH],
                    start=(kk == 0),
                    stop=(kk == NT - 1),
                )

    def close_last_chains(cc):
        for sub in range(2):
            nc.vector.tensor_copy(ot_sb[2 * cc + sub][:, 0:NH], last_chains[sub])

    # Steady-state weave per block c:
    #   k step:  S(c,k), then
    #     c==0:  masks k+2, adj dma k+4, V chunk k
    #     c>=1:  k<4: PV piece k of pair c-1;  k>=4: finalize q-chunks of c-1
    #            plus one QK(c+1) projection unit at odd k
    for c in range(NPAIR):
        for k in range(NT):
            if c == 0:
                emit_mask(k)
            emit_s_step(c, k)
            if c == 0:
                if k >= 3:
                    emit_v(k - 3)
                if k == NT - 1:
                    for i in range(NT - 3, NT):
                        emit_v(i)
            else:
                if k < 4:
                    emit_pv_piece(c - 1, k)
                else:
                    emit_fin_qc(c - 1, 2 * (k - 4))
                    emit_fin_qc(c - 1, 2 * (k - 4) + 1)
                if k >= 4 and c + 1 < DT:
                    wname, dst, bias_sb = (
                        ("wq", qt, bq_sb) if k < 6 else ("wk", kt, bk_sb)
                    )
                    emit_qk_chunk(c + 1, wname, dst, bias_sb, k % 2)
                if c == NPAIR - 1 and k >= 4:
                    if k == 4:
                        open_last_chains(c)
                        feed_last_chains(c, range(4))
                    else:
                        feed_last_chains(c, [k - 1])
    # drain: q-half-0 fins (qc 0-3) only need the incrementally-fed chains,
    # so they overlap the q-half-1 PV pieces on the PE queue
    feed_last_chains(NPAIR - 1, [NT - 1])
    close_last_chains(NPAIR - 1)
    emit_pv_piece(NPAIR - 1, 1)
    emit_fin_qc(NPAIR - 1, 0)
    emit_fin_qc(NPAIR - 1, 1)
    emit_pv_piece(NPAIR - 1, 3)
    emit_fin_qc(NPAIR - 1, 2)
    emit_fin_qc(NPAIR - 1, 3)
    for qc in range(4, NT):
        emit_fin_qc(NPAIR - 1, qc)


def build_nc():
    nc = bacc.Bacc("TRN2", target_bir_lowering=False, debug=False, num_devices=B)
    x_d = nc.dram_tensor("x", [N, D], BF16, kind="ExternalInput").ap()
    adj_d = nc.dram_tensor("adj", [N, N], BF16, kind="ExternalInput").ap()
    w_d = {
        "wq": nc.dram_tensor("wq", [D, D], BF16, kind="ExternalInput").ap(),
        "wk": nc.dram_tensor("wk", [D, D], BF16, kind="ExternalInput").ap(),
        "wv": nc.dram_tensor("wv", [D, D], BF16, kind="ExternalInput").ap(),
    }
    b_d = {
        "bq": nc.dram_tensor("bq", [D], F32, kind="ExternalInput").ap(),
        "bk": nc.dram_tensor("bk", [D], F32, kind="ExternalInput").ap(),
        "bv": nc.dram_tensor("bv", [D], F32, kind="ExternalInput").ap(),
    }
    beta_d = nc.dram_tensor("beta", [1, 1], F32, kind="ExternalInput").ap()
    out_d = nc.dram_tensor("out", [N, D], BF16, kind="ExternalOutput").ap()
    with tile.TileContext(nc) as tc, ExitStack() as ctx:
        _emit(tc, ctx, x_d, adj_d, w_d, b_d, beta_d, out_d)
    nc.compile()
    return nc


_CACHE = {}


def _get_nc():
    if "nc" not in _CACHE:
        _CACHE["nc"] = build_nc()
    return _CACHE["nc"]


def make_in_maps(input_graph, adj, Wq, bq, Wk, bk, Wv, bv, beta):
    import ml_dtypes

    f = lambda a: np.ascontiguousarray(np.asarray(a), dtype=np.float32)
    h = lambda a: np.ascontiguousarray(np.asarray(a, dtype=np.float32)).astype(
        ml_dtypes.bfloat16
    )
    wq, wk, wv = h(Wq), h(Wk), h(Wv)
    bqa, bka, bva = f(bq), f(bk), f(bv)
    beta_a = f(beta).reshape(1, 1)
    ig, ad = h(input_graph), h(adj)
    return [
        {
            "x": ig[b], "adj": ad[b],
            "wq": wq, "wk": wk, "wv": wv,
            "bq": bqa, "bk": bka, "bv": bva,
            "beta": beta_a,
        }
        for b in range(B)
    ]


def run_hw(in_maps, **kwargs):
    nc = _get_nc()
    return run_bass_kernel_spmd(nc, in_maps, list(range(B)), **kwargs)


def kernel(input_graph, adj, Wq, bq, Wk, bk, Wv, bv, beta):
    in_maps = make_in_maps(input_graph, adj, Wq, bq, Wk, bk, Wv, bv, beta)
    res = run_hw(in_maps)
    return np.stack(
        [np.asarray(res.results[i]["out"], dtype=np.float32) for i in range(B)], axis=0
    )
